# revision 23
# baseline (speedup 1.0000x reference)
"""Self-contained Trainium2 Bass kernel for the 2-layer GAT problem
(nn_GAT_85899346613): 100000 nodes, 800000 edges, F_in=128, layer1 8 heads x 16,
layer2 1 head x 2. Distributes across 8 NeuronCores by dst-node ownership.

Single fused SPMD launch:
  A: node phase  h1 = x @ W1, attention terms u, v   (per-core local nodes)
     -> on-device AllGather of T1=[h1|u]
  B: layer-1 edge phase (gather src rows, segment softmax via fp8 one-hot
     mask matmuls into PSUM) + ELU + layer-2 node values z, u2, v2
     -> on-device AllGather of Tz=[z0 z1 u2 v2]
  C: layer-2 edge phase -> output [N, 2]

All edge-structure constants (one-hot masks, gather offsets) and weights stay
device-resident across calls; only x is re-uploaded when it changes.
"""
import sys
if "/opt/trn_rl_repo" not in sys.path:
    sys.path.insert(0, "/opt/trn_rl_repo")
import numpy as np
import ml_dtypes
import concourse.bass as bass
import concourse.bacc as bacc
import concourse.mybir as mybir
import concourse.tile as tile

try:
    import jax as _jax
    _jax.config.update("jax_compilation_cache_dir",
                       "/tmp/gat_jax_compile_cache")
    _jax.config.update("jax_persistent_cache_min_compile_time_secs", 1.0)
    _jax.config.update("jax_persistent_cache_min_entry_size_bytes", 0)
except Exception:
    pass

P = 128
F32 = mybir.dt.float32
BF16 = mybir.dt.bfloat16
FP8 = mybir.dt.float8e4
I32 = mybir.dt.int32
AF = mybir.ActivationFunctionType
ALU = mybir.AluOpType
NPBF16 = ml_dtypes.bfloat16


# ---------------------------------------------------------------------------
# Host-side edge preprocessing (cached across calls keyed on edge_index)
# ---------------------------------------------------------------------------

def prepare(edge_index: np.ndarray, N: int, n_cores: int = 8, nb: int = None):
    E = edge_index.shape[1]
    src0 = edge_index[0].astype(np.int64)
    dst0 = edge_index[1].astype(np.int64)

    nodes_per_core = -(-N // n_cores)
    if nb is None:
        nb = -(-nodes_per_core // P)
    NB = nb
    NL = NB * P

    # degree over non-self edges (self loops handled separately)
    deg = np.bincount(dst0, minlength=N)

    # ---- snake-deal nodes (degree desc) across cores, then blocks ----
    order = np.argsort(-deg, kind="stable")
    pos = np.arange(N)
    rnd = pos // n_cores
    idx = pos % n_cores
    lane = np.where((rnd % 2) == 0, idx, n_cores - 1 - idx)
    node_core = np.empty(N, np.int32)
    node_core[order] = lane.astype(np.int32)

    node_block = np.empty(N, np.int32)
    node_slot = np.empty(N, np.int32)
    for c in range(n_cores):
        nodes_c = order[lane == c]
        m = nodes_c.shape[0]
        assert m <= NB * P, (c, m)
        pos = np.arange(m)
        rnd = pos // NB
        idx = pos % NB
        blk = np.where((rnd % 2) == 0, idx, NB - 1 - idx).astype(np.int32)
        node_block[nodes_c] = blk
        node_slot[nodes_c] = rnd.astype(np.int32)
        assert rnd.max() < P

    new_id = node_core.astype(np.int64) * NL + node_block * P + node_slot
    NROWS = n_cores * NL
    old_of_new = np.full(NROWS, -1, np.int64)
    old_of_new[new_id] = np.arange(N)

    # ---- per-core edge layout (non-self edges only; self loops = chunk 0) ----
    src_new = new_id[src0]
    e_core = node_core[dst0]
    e_block = node_block[dst0]
    e_dslot = node_slot[dst0]

    counts = np.zeros((n_cores, NB), np.int64)
    for c in range(n_cores):
        sel = e_core == c
        counts[c] = np.bincount(e_block[sel], minlength=NB)
    # chunks per block: 1 self chunk + ceil(max_edges/128)
    kb = 1 + np.ceil(counts.max(axis=0) / P).astype(np.int64)
    chunk_block = np.repeat(np.arange(NB), kb)
    NCHUNK = int(kb.sum())
    block_start_chunk = np.concatenate([[0], np.cumsum(kb)])[:-1]

    # n_nodes per (core, block) for self-chunk masks
    nnodes = np.zeros((n_cores, NB), np.int64)
    for c in range(n_cores):
        nnodes[c] = np.bincount(node_block[node_core == c], minlength=NB)

    cores = []
    for c in range(n_cores):
        sel = np.nonzero(e_core == c)[0]
        eb = e_block[sel]
        o = np.argsort(eb, kind="stable")
        sel, eb = sel[o], eb[o]
        starts = np.concatenate([[0], np.cumsum(np.bincount(eb, minlength=NB))])[:-1]
        within = np.arange(sel.shape[0]) - starts[eb]
        slots = (block_start_chunk[eb] + 1) * P + within  # +1: skip self chunk

        slot_src = np.zeros(NCHUNK * P, np.int32)
        slot_d = np.full(NCHUNK * P, -1, np.int32)
        slot_src[slots] = src_new[sel].astype(np.int32)
        slot_d[slots] = e_dslot[sel].astype(np.int32)
        # self chunks: slot p -> node p of block (if present)
        for b in range(NB):
            k0 = block_start_chunk[b]
            nn = nnodes[c, b]
            sl = np.arange(nn)
            slot_d[k0 * P + sl] = sl
            slot_src[k0 * P + sl] = c * NL + b * P + sl  # unused (plain load)

        sd = slot_d.reshape(NCHUNK, P)
        sdT = np.ascontiguousarray(sd.T).astype(np.int32)        # [P, NCHUNK]
        offd = chunk_block[:, None] * P + np.maximum(sd, 0)      # [NCHUNK, P]
        offdT = np.ascontiguousarray(offd.T).astype(np.int32)    # [P, NCHUNK]
        offs_src = slot_src.reshape(NCHUNK, P).T.copy()          # [P, NCHUNK]
        cores.append(dict(offs_src=offs_src, sdT=sdT, offd=offdT))

    return dict(
        cores=cores, new_id=new_id, old_of_new=old_of_new,
        NROWS=NROWS, NCHUNK=NCHUNK, kb=kb, chunk_block=chunk_block,
        block_start_chunk=block_start_chunk,
        node_core=node_core, NB=NB, NL=NL, n_cores=n_cores,
    )


def host_precompute(W1, att_src1, att_dst1, W2, att_src2, att_dst2):
    H, C = att_src1.shape
    Wu = (W1.reshape(-1, H, C) * att_src1[None]).sum(-1)   # [F_in, H]
    Wv = (W1.reshape(-1, H, C) * att_dst1[None]).sum(-1)
    WuWv = np.concatenate([Wu, Wv], axis=1).astype(np.float32)  # [F_in, 16]
    Wu2 = W2 @ att_src2[0]     # [HC]
    Wv2 = W2 @ att_dst2[0]
    Wz2 = np.stack([W2[:, 0], W2[:, 1], Wu2, Wv2], axis=1).astype(np.float32)  # [HC, 4]
    return WuWv, Wz2


# ---------------------------------------------------------------------------
# Fused single-launch Bass kernel
# ---------------------------------------------------------------------------

def build_fused(kb, n_cores: int):
    NB = len(kb)
    NCHUNK = int(np.sum(kb))
    NL = NB * P
    NROWS = n_cores * NL
    T1W = 136  # [h1 (128) | u (8)]

    nc = bacc.Bacc("TRN2", target_bir_lowering=False, debug=False,
                   num_devices=n_cores)
    t_xT = nc.dram_tensor("xT", [P, NL], F32, kind="ExternalInput")
    t_W1 = nc.dram_tensor("W1", [P, P], F32, kind="ExternalInput")
    t_Wuv = nc.dram_tensor("WuWv", [P, 16], F32, kind="ExternalInput")
    t_B1 = nc.dram_tensor("B1", [1, P], F32, kind="ExternalInput")
    t_Wz2 = nc.dram_tensor("Wz2", [P, 4], BF16, kind="ExternalInput")
    t_ID = nc.dram_tensor("ID", [P, P], BF16, kind="ExternalInput")
    t_SD = nc.dram_tensor("SD", [P, NCHUNK], I32, kind="ExternalInput")
    t_OFFS = nc.dram_tensor("OFFS", [P, NCHUNK], I32, kind="ExternalInput")
    t_OFFD = nc.dram_tensor("OFFD", [P, NCHUNK], I32, kind="ExternalInput")
    t_OUT = nc.dram_tensor("OUT", [P, NROWS * 2 // P], BF16, kind="ExternalOutput")

    chunk_block = np.repeat(np.arange(NB), kb)
    csum = np.concatenate([[0], np.cumsum(kb)])
    block_first, block_last = csum[:-1], csum[1:] - 1
    RG = [list(range(n_cores))]

    with tile.TileContext(nc) as tc:
        with tc.tile_pool(name="const", bufs=1) as cpool, \
             tc.tile_pool(name="dram", bufs=1, space="DRAM") as dram:
            W1sb = cpool.tile([P, P], F32, tag="W1sb")
            nc.sync.dma_start(out=W1sb[:], in_=t_W1.ap())
            Wuvsb = cpool.tile([P, 16], F32, tag="Wuvsb")
            nc.sync.dma_start(out=Wuvsb[:], in_=t_Wuv.ap())
            b1row = cpool.tile([1, P], F32, tag="b1row")
            nc.sync.dma_start(out=b1row[:], in_=t_B1.ap())
            Wz2sb = cpool.tile([P, 4], BF16, tag="Wz2sb")
            nc.sync.dma_start(out=Wz2sb[:], in_=t_Wz2.ap())
            idsb = cpool.tile([P, P], BF16, tag="idsb")
            nc.sync.dma_start(out=idsb[:], in_=t_ID.ap())
            sd_t = cpool.tile([P, NCHUNK], I32, tag="sd_t")
            nc.sync.dma_start(out=sd_t[:], in_=t_SD.ap())
            offs_t = cpool.tile([P, NCHUNK], I32, tag="offs_t")
            nc.sync.dma_start(out=offs_t[:], in_=t_OFFS.ap())
            offd_t = cpool.tile([P, NCHUNK], I32, tag="offd_t")
            nc.sync.dma_start(out=offd_t[:], in_=t_OFFD.ap())
            onesb = cpool.tile([1, P], F32, tag="onesb")
            nc.gpsimd.memset(onesb[:], 1.0)
            iota_t = cpool.tile([P, P], I32, tag="iota_t")
            nc.gpsimd.iota(iota_t[:], pattern=[[1, P]], base=0,
                           channel_multiplier=0)
            b1bc = cpool.tile([P, P], BF16, tag="b1bc")

            T1_loc = dram.tile([NL, T1W], BF16, tag="T1_loc")
            T1_full = dram.tile([NROWS, T1W], BF16, addr_space="Shared",
                                tag="T1_full")
            TVd = dram.tile([NL, 8], BF16, tag="TVd")
            Tz_loc = dram.tile([NL, 4], F32, tag="Tz_loc")
            Tz_full = dram.tile([NROWS, 4], F32, addr_space="Shared",
                                tag="Tz_full")
            OUT_loc = dram.tile([NL, 2], F32, tag="OUT_loc")
            OUT_full = dram.tile([NROWS, 2], F32, addr_space="Shared",
                                 tag="OUT_full")

            # ---------------- phase A: node values ----------------
            with tc.tile_pool(name="psB1", bufs=1, space="PSUM") as psb1:
                pb1 = psb1.tile([P, P], F32, tag="pb1")
                nc.tensor.matmul(pb1[:], lhsT=onesb[:], rhs=b1row[:],
                                 start=True, stop=True)
                nc.scalar.activation(b1bc[:], pb1[:], AF.Copy)
            with tc.tile_pool(name="sbA", bufs=3) as sbA, \
                 tc.tile_pool(name="psA", bufs=4, space="PSUM") as psA:
                for t in range(NB):
                    xTt = sbA.tile([P, P], F32, tag="xT")
                    nc.sync.dma_start(out=xTt[:],
                                      in_=t_xT.ap()[:, t * P:(t + 1) * P])
                    ph = psA.tile([P, P], F32, tag="ph")
                    nc.tensor.matmul(ph[:], lhsT=xTt[:], rhs=W1sb[:],
                                     start=True, stop=True)
                    puv = psA.tile([P, 16], F32, tag="puv")
                    nc.tensor.matmul(puv[:], lhsT=xTt[:], rhs=Wuvsb[:],
                                     start=True, stop=True)
                    hrow = sbA.tile([P, T1W], BF16, tag="hrow")
                    nc.scalar.activation(hrow[:, 0:P], ph[:], AF.Copy)
                    nc.vector.tensor_copy(hrow[:, P:P + 8], puv[:, 0:8])
                    vrow = sbA.tile([P, 8], BF16, tag="vrow")
                    nc.vector.tensor_copy(vrow[:], puv[:, 8:16])
                    nc.sync.dma_start(out=T1_loc[t * P:(t + 1) * P, :],
                                      in_=hrow[:])
                    nc.sync.dma_start(out=TVd[t * P:(t + 1) * P, :],
                                      in_=vrow[:])

            nc.gpsimd.collective_compute(
                "AllGather", ALU.bypass, replica_groups=RG,
                ins=[T1_loc.opt()], outs=[T1_full.opt()])

            # ---------------- phase B: layer-1 edges ----------------
            GW = 8
            with tc.tile_pool(name="gatB", bufs=3) as gat, \
                 tc.tile_pool(name="winB", bufs=3) as win, \
                 tc.tile_pool(name="blkB", bufs=3) as blk, \
                 tc.tile_pool(name="accB", bufs=4, space="PSUM") as accp, \
                 tc.tile_pool(name="ps2B", bufs=2, space="PSUM") as ps2:
                psum_of_block = {}
                for w0 in range(0, NCHUNK, GW):
                    w1 = min(w0 + GW, NCHUNK)
                    nw = w1 - w0
                    s_gen = win.tile([P, GW * P], BF16, tag="sg")
                    nc.vector.tensor_tensor(
                        s_gen[:, 0:nw * P].rearrange("p (w d) -> p w d", d=P),
                        sd_t[:, w0:w1].unsqueeze(2).to_broadcast([P, nw, P]),
                        iota_t[:].unsqueeze(1).to_broadcast([P, nw, P]),
                        op=ALU.is_equal)
                    g = gat.tile([P, GW, T1W], BF16, tag="g")
                    vg = gat.tile([P, GW, 8], BF16, tag="vg")
                    for k in range(w0, w1):
                        b = chunk_block[k]
                        if k == block_first[b]:
                            nc.sync.dma_start(
                                out=g[:, k - w0, :],
                                in_=T1_loc[b * P:(b + 1) * P, :])
                            nc.sync.dma_start(
                                out=vg[:, k - w0, :],
                                in_=TVd[b * P:(b + 1) * P, :])
                        else:
                            nc.gpsimd.indirect_dma_start(
                                out=g[:, k - w0, :], out_offset=None,
                                in_=T1_full[:],
                                in_offset=bass.IndirectOffsetOnAxis(
                                    ap=offs_t[:, k:k + 1], axis=0))
                            nc.gpsimd.indirect_dma_start(
                                out=vg[:, k - w0, :], out_offset=None,
                                in_=TVd[:],
                                in_offset=bass.IndirectOffsetOnAxis(
                                    ap=offd_t[:, k:k + 1], axis=0))
                    # e = u + v ; lrelu ; exp
                    e_t = win.tile([P, GW, 8], F32, tag="e")
                    nc.vector.tensor_add(
                        e_t[:, 0:nw, :], g[:, 0:nw, P:P + 8], vg[:, 0:nw, :])
                    nc.vector.scalar_tensor_tensor(
                        e_t[:, 0:nw, :], in0=e_t[:, 0:nw, :], scalar=0.2,
                        in1=e_t[:, 0:nw, :], op0=ALU.mult, op1=ALU.max)
                    ex_t = win.tile([P, GW, 8], BF16, tag="ex")
                    nc.scalar.activation(ex_t[:, 0:nw, :], e_t[:, 0:nw, :],
                                         AF.Exp)
                    m_t = win.tile([P, GW, T1W], BF16, tag="m")
                    exb = ex_t[:, 0:nw, :].unsqueeze(3).to_broadcast(
                        [P, nw, 8, 16])
                    nc.vector.tensor_mul(
                        m_t[:, 0:nw, 0:P].rearrange(
                            "p w (h c) -> p w h c", c=16),
                        g[:, 0:nw, 0:P].rearrange(
                            "p w (h c) -> p w h c", c=16),
                        exb)
                    nc.vector.tensor_copy(m_t[:, 0:nw, P:P + 8],
                                          ex_t[:, 0:nw, :])
                    for k in range(w0, w1):
                        b = chunk_block[k]
                        if k == block_first[b]:
                            psum_of_block[b] = accp.tile(
                                [P, T1W], F32, tag="acc", name=f"acc{b}")
                        nc.tensor.matmul(
                            psum_of_block[b][:],
                            lhsT=s_gen[:, (k - w0) * P:(k - w0 + 1) * P],
                            rhs=m_t[:, k - w0, :],
                            start=(k == block_first[b]),
                            stop=(k == block_last[b]))
                        if k == block_last[b]:
                            acc = psum_of_block.pop(b)
                            _finish_block_B(nc, blk, ps2, acc, Wz2sb, idsb,
                                            b1bc, Tz_loc, b)

            nc.gpsimd.collective_compute(
                "AllGather", ALU.bypass, replica_groups=RG,
                ins=[Tz_loc.opt()], outs=[Tz_full.opt()])

            # ---------------- phase C: layer-2 edges ----------------
            GW2 = 16
            with tc.tile_pool(name="gatC", bufs=3) as gat, \
                 tc.tile_pool(name="winC", bufs=3) as win, \
                 tc.tile_pool(name="blkC", bufs=3) as blk, \
                 tc.tile_pool(name="accC", bufs=4, space="PSUM") as accp:
                psum_of_block = {}
                for w0 in range(0, NCHUNK, GW2):
                    w1 = min(w0 + GW2, NCHUNK)
                    nw = w1 - w0
                    s_gen = win.tile([P, GW2 * P], BF16, tag="sg")
                    nc.vector.tensor_tensor(
                        s_gen[:, 0:nw * P].rearrange("p (w d) -> p w d", d=P),
                        sd_t[:, w0:w1].unsqueeze(2).to_broadcast([P, nw, P]),
                        iota_t[:].unsqueeze(1).to_broadcast([P, nw, P]),
                        op=ALU.is_equal)
                    zg = gat.tile([P, GW2, 4], F32, tag="zg")
                    vg2 = gat.tile([P, GW2, 4], F32, tag="vg2")
                    for k in range(w0, w1):
                        b = chunk_block[k]
                        if k == block_first[b]:
                            nc.sync.dma_start(
                                out=zg[:, k - w0, :],
                                in_=Tz_loc[b * P:(b + 1) * P, :])
                            nc.sync.dma_start(
                                out=vg2[:, k - w0, :],
                                in_=Tz_loc[b * P:(b + 1) * P, :])
                        else:
                            nc.gpsimd.indirect_dma_start(
                                out=zg[:, k - w0, :], out_offset=None,
                                in_=Tz_full[:],
                                in_offset=bass.IndirectOffsetOnAxis(
                                    ap=offs_t[:, k:k + 1], axis=0))
                            nc.gpsimd.indirect_dma_start(
                                out=vg2[:, k - w0, :], out_offset=None,
                                in_=Tz_loc[:],
                                in_offset=bass.IndirectOffsetOnAxis(
                                    ap=offd_t[:, k:k + 1], axis=0))
                    e_t = win.tile([P, GW2], F32, tag="e")
                    nc.vector.tensor_add(e_t[:, 0:nw], zg[:, 0:nw, 2],
                                         vg2[:, 0:nw, 3])
                    nc.vector.scalar_tensor_tensor(
                        e_t[:, 0:nw], in0=e_t[:, 0:nw], scalar=0.2,
                        in1=e_t[:, 0:nw], op0=ALU.mult, op1=ALU.max)
                    ex_t = win.tile([P, GW2], F32, tag="ex")
                    nc.scalar.activation(ex_t[:, 0:nw], e_t[:, 0:nw], AF.Exp)
                    m_t = win.tile([P, GW2, 3], BF16, tag="m")
                    nc.vector.tensor_mul(
                        m_t[:, 0:nw, 0:2], zg[:, 0:nw, 0:2],
                        ex_t[:, 0:nw].unsqueeze(2).to_broadcast([P, nw, 2]))
                    nc.vector.tensor_copy(m_t[:, 0:nw, 2], ex_t[:, 0:nw])
                    for k in range(w0, w1):
                        b = chunk_block[k]
                        if k == block_first[b]:
                            psum_of_block[b] = accp.tile(
                                [P, 3], F32, tag="acc", name=f"acc{b}")
                        nc.tensor.matmul(
                            psum_of_block[b][:],
                            lhsT=s_gen[:, (k - w0) * P:(k - w0 + 1) * P],
                            rhs=m_t[:, k - w0, :],
                            start=(k == block_first[b]),
                            stop=(k == block_last[b]))
                        if k == block_last[b]:
                            acc = psum_of_block.pop(b)
                            s_eps = blk.tile([P, 1], F32, tag="seps")
                            nc.vector.tensor_scalar_add(s_eps[:],
                                                        acc[:, 2:3], 1e-16)
                            rcp = blk.tile([P, 1], F32, tag="rcp")
                            nc.vector.reciprocal(rcp[:], s_eps[:])
                            orow = blk.tile([P, 2], F32, tag="orow")
                            nc.vector.tensor_mul(orow[:], acc[:, 0:2],
                                                 rcp[:].to_broadcast([P, 2]))
                            nc.sync.dma_start(
                                out=OUT_loc[b * P:(b + 1) * P, :],
                                in_=orow[:])

            nc.gpsimd.collective_compute(
                "AllGather", ALU.bypass, replica_groups=RG,
                ins=[OUT_loc.opt()], outs=[OUT_full.opt()])
            # cast f32 -> bf16 through SBUF to halve the D2H payload
            QF = NROWS * 2 // P
            with tc.tile_pool(name="cast", bufs=2) as cast:
                of32 = cast.tile([P, QF], F32, tag="of32")
                nc.sync.dma_start(
                    out=of32[:],
                    in_=OUT_full.rearrange("(p q) c -> p (q c)", p=P))
                obf = cast.tile([P, QF], BF16, tag="obf")
                nc.vector.tensor_copy(obf[:], of32[:])
                nc.sync.dma_start(out=t_OUT.ap(), in_=obf[:])
    nc.compile()
    return nc


def _finish_block_B(nc, blk, ps2, acc, Wz2sb, idsb, b1bc, Tz_loc, b):
    s_eps = blk.tile([P, 8], F32, tag="seps")
    nc.vector.tensor_scalar_add(s_eps[:], acc[:, P:P + 8], 1e-16)
    rcp = blk.tile([P, 8], F32, tag="rcp")
    nc.vector.reciprocal(rcp[:], s_eps[:])
    h2 = blk.tile([P, P], BF16, tag="h2")
    nc.vector.tensor_mul(
        h2[:].rearrange("p (h c) -> p h c", c=16),
        acc[:, 0:P].rearrange("p (h c) -> p h c", c=16),
        rcp[:].unsqueeze(2).to_broadcast([P, 8, 16]))
    nc.vector.tensor_add(h2[:], h2[:], b1bc[:])
    # ELU = max(h2,0) + min(exp(h2)-1, 0)
    ex1 = blk.tile([P, P], BF16, tag="elu1")
    nc.scalar.activation(ex1[:], h2[:], AF.Exp)
    nc.vector.tensor_scalar(ex1[:], in0=ex1[:], scalar1=-1.0, scalar2=0.0,
                            op0=ALU.add, op1=ALU.min)
    h2e = blk.tile([P, P], BF16, tag="h2e")
    nc.vector.scalar_tensor_tensor(h2e[:], in0=h2[:], scalar=0.0, in1=ex1[:],
                                   op0=ALU.max, op1=ALU.add)
    pt = ps2.tile([P, P], BF16, tag="ps2")
    nc.tensor.transpose(pt[:], h2e[:], idsb[:])
    h2T = blk.tile([P, P], BF16, tag="h2T")
    nc.scalar.activation(h2T[:], pt[:], AF.Copy)
    pz = ps2.tile([P, 4], F32, tag="ps2")
    nc.tensor.matmul(pz[:], lhsT=h2T[:], rhs=Wz2sb[:], start=True, stop=True)
    zrow = blk.tile([P, 4], F32, tag="zrow")
    nc.vector.tensor_copy(zrow[:], pz[:])
    nc.sync.dma_start(out=Tz_loc[b * P:(b + 1) * P, :], in_=zrow[:])


# ---------------------------------------------------------------------------
# Persistent PJRT runner: compile once, keep constants device-resident
# ---------------------------------------------------------------------------

class FusedRunner:
    def __init__(self, nc, n_cores: int):
        import jax
        from jax.sharding import Mesh, NamedSharding, PartitionSpec
        from jax.experimental.shard_map import shard_map
        from concourse import bass2jax
        bass2jax.install_neuronx_cc_hook()
        self.jax = jax
        self.nc = nc
        self.n_cores = n_cores

        pid_name = (nc.partition_id_tensor.name
                    if nc.partition_id_tensor is not None else None)
        in_names, out_names, out_avals = [], [], []
        for alloc in nc.m.functions[0].allocations:
            if not isinstance(alloc, mybir.MemoryLocationSet):
                continue
            name = alloc.memorylocations[0].name
            if alloc.kind == "ExternalInput":
                if name != pid_name:
                    in_names.append(name)
            elif alloc.kind == "ExternalOutput":
                assert alloc.tensor_shape is not None
                out_names.append(name)
                out_avals.append(jax.core.ShapedArray(
                    tuple(alloc.tensor_shape), mybir.dt.np(alloc.dtype)))
        self.in_names = list(in_names)
        self.out_names = out_names

        all_names = list(in_names) + list(out_names)
        if pid_name is not None:
            all_names.append(pid_name)
        dbg_name = nc.dbg_addr.name if nc.dbg_addr is not None else None
        if dbg_name is not None and dbg_name in self.in_names:
            raise RuntimeError("debug kernels not supported by FusedRunner")

        def _body(*args):
            operands = list(args)
            if pid_name is not None:
                operands.append(bass2jax.partition_id_tensor())
            outs = bass2jax._bass_exec_p.bind(
                *operands,
                out_avals=tuple(out_avals),
                in_names=tuple(all_names),
                out_names=tuple(out_names),
                lowering_input_output_aliases=(),
                sim_require_finite=False,
                sim_require_nnan=False,
                nc=nc,
            )
            return tuple(outs)

        devices = jax.devices()[:n_cores]
        assert len(devices) == n_cores
        self.mesh = Mesh(np.asarray(devices), ("core",))
        self.sharding = NamedSharding(self.mesh, PartitionSpec("core"))
        n_args = len(in_names) + len(out_avals)
        in_specs = (PartitionSpec("core"),) * n_args
        out_specs = (PartitionSpec("core"),) * len(out_names)
        self._fn = jax.jit(shard_map(
            _body, mesh=self.mesh, in_specs=in_specs, out_specs=out_specs,
            check_rep=False))
        # persistent (never-donated) zero buffers for the output params
        self._zeros = [
            jax.device_put(
                np.zeros((n_cores * aval.shape[0], *aval.shape[1:]),
                         aval.dtype), self.sharding)
            for aval in out_avals
        ]
        # AOT-compile with the bass effect suppressed (C++ fast-path
        # dispatch); fall back to the plain jit on any failure.
        in_avals = []
        for alloc in nc.m.functions[0].allocations:
            if not isinstance(alloc, mybir.MemoryLocationSet):
                continue
            name = alloc.memorylocations[0].name
            if alloc.kind == "ExternalInput" and name in self.in_names:
                in_avals.append((tuple(alloc.tensor_shape),
                                 mybir.dt.np(alloc.dtype)))
        try:
            sds = [jax.ShapeDtypeStruct((n_cores * s[0], *s[1:]), dt,
                                        sharding=self.sharding)
                   for s, dt in in_avals]
            sds += [jax.ShapeDtypeStruct(z.shape, z.dtype,
                                         sharding=self.sharding)
                    for z in self._zeros]
            from concourse.bass2jax import fast_dispatch_compile
            self._fast = fast_dispatch_compile(
                lambda: self._fn.lower(*sds).compile())
        except Exception:
            self._fast = None
        self.bufs = {}

    def put(self, name: str, per_core: list):
        """Upload per-core arrays (list of n_cores ndarrays) once."""
        glob = np.concatenate([np.ascontiguousarray(a) for a in per_core], axis=0)
        self.bufs[name] = self.jax.device_put(glob, self.sharding)

    def dispatch(self):
        """Async launch; returns output jax arrays (futures)."""
        args = [self.bufs[nm] for nm in self.in_names] + self._zeros
        if self._fast is not None:
            try:
                return self._fast(*args)
            except Exception:
                self._fast = None
        return self._fn(*args)

    def fetch(self, outs):
        """Fetch outputs; OUT is replicated across cores, so pull only the
        first shard (one D2H round trip)."""
        res = {}
        for nm, arr in zip(self.out_names, outs):
            s0 = arr.addressable_shards[0]
            try:
                s0.data.copy_to_host_async()
            except Exception:
                pass
            res[nm] = np.asarray(s0.data)
        return res

    def run(self):
        return self.fetch(self.dispatch())


# ---------------------------------------------------------------------------
# kernel() entry with content-addressed caching
# ---------------------------------------------------------------------------

_STATE = {}


def _same(a, b):
    return (b is not None and a.shape == b.shape and a.dtype == b.dtype
            and np.array_equal(a, b))


def kernel(x, edge_index, W1, att_src1, att_dst1, b1, W2, att_src2, att_dst2, b2):
    x = np.ascontiguousarray(np.asarray(x, dtype=np.float32))
    edge_index = np.ascontiguousarray(np.asarray(edge_index))
    W1 = np.asarray(W1, np.float32); W2 = np.asarray(W2, np.float32)
    att_src1 = np.asarray(att_src1, np.float32)
    att_dst1 = np.asarray(att_dst1, np.float32)
    att_src2 = np.asarray(att_src2, np.float32)
    att_dst2 = np.asarray(att_dst2, np.float32)
    b1 = np.asarray(b1, np.float32); b2 = np.asarray(b2, np.float32)
    N = x.shape[0]
    n_cores = 8

    st = _STATE
    # Optimistic dispatch: reuse the run pre-launched at the end of the
    # previous call if there is one, else launch now with the currently
    # device-resident inputs; verify cache hits while it runs.
    spec_outs = None
    if "runner" in st:
        try:
            spec_outs = st["runner"].dispatch()
        except Exception:
            spec_outs = None

    # --- edge-structure cache (masks, offsets, runner) ---
    if not (_same(edge_index, st.get("edges")) and st.get("N") == N):
        spec_outs = None
        pr = prepare(edge_index, N, n_cores)
        nc = build_fused(pr["kb"], n_cores)
        runner = FusedRunner(nc, n_cores)
        runner.put("SD", [co["sdT"] for co in pr["cores"]])
        runner.put("OFFS", [co["offs_src"] for co in pr["cores"]])
        runner.put("OFFD", [co["offd"] for co in pr["cores"]])
        st.clear()
        st.update(edges=edge_index.copy(), N=N, pr=pr, runner=runner,
                  weights=None, x=None)
    pr, runner = st["pr"], st["runner"]

    # --- weight cache ---
    wpack = np.concatenate([W1.ravel(), att_src1.ravel(), att_dst1.ravel(),
                            b1.ravel(), W2.ravel(), att_src2.ravel(),
                            att_dst2.ravel(), b2.ravel()])
    if not _same(wpack, st.get("weights")):
        spec_outs = None
        WuWv, Wz2 = host_precompute(W1, att_src1, att_dst1, W2, att_src2,
                                    att_dst2)
        ident = np.eye(P, dtype=NPBF16)
        runner.put("W1", [W1.astype(np.float32)] * n_cores)
        runner.put("WuWv", [WuWv] * n_cores)
        runner.put("B1", [b1.reshape(1, P).astype(np.float32)] * n_cores)
        runner.put("Wz2", [Wz2.astype(NPBF16)] * n_cores)
        runner.put("ID", [ident] * n_cores)
        st["weights"] = wpack.copy()
        st["b2"] = b2.copy()

    # --- x cache ---
    if not _same(x, st.get("x")):
        spec_outs = None
        NL, NROWS = pr["NL"], pr["NROWS"]
        xg = np.zeros((NROWS, x.shape[1]), np.float32)
        xg[pr["new_id"]] = x
        xTs = [np.ascontiguousarray(xg[c * NL:(c + 1) * NL].T)
               for c in range(n_cores)]
        runner.put("xT", xTs)
        st["x"] = x.copy()

    if spec_outs is None:
        spec_outs = runner.dispatch()
    try:
        outs = runner.fetch(spec_outs)
    except Exception:
        # transient device/relay error: back off and retry a fresh launch
        import time as _time
        outs = None
        for delay in (0.5, 2.0, 5.0):
            _time.sleep(delay)
            try:
                outs = runner.fetch(runner.dispatch())
                break
            except Exception:
                continue
        if outs is None:
            outs = runner.fetch(runner.dispatch())
    out_full = np.asarray(outs["OUT"], dtype=np.float32).reshape(pr["NROWS"], 2)
    out = out_full[pr["new_id"]]
    return np.ascontiguousarray((out + st["b2"][None, :]).astype(np.float32))


# revision 24
# speedup vs baseline: 2.7271x; 2.7271x over previous
"""Self-contained Trainium2 Bass kernel for the 2-layer GAT problem
(nn_GAT_85899346613): 100000 nodes, 800000 edges, F_in=128, layer1 8 heads x 16,
layer2 1 head x 2. Distributes across 8 NeuronCores by dst-node ownership.

Single fused SPMD launch:
  A: node phase  h1 = x @ W1, attention terms u, v   (per-core local nodes)
     -> on-device AllGather of T1=[h1|u]
  B: layer-1 edge phase (gather src rows, segment softmax via fp8 one-hot
     mask matmuls into PSUM) + ELU + layer-2 node values z, u2, v2
     -> on-device AllGather of Tz=[z0 z1 u2 v2]
  C: layer-2 edge phase -> output [N, 2]

All edge-structure constants (one-hot masks, gather offsets) and weights stay
device-resident across calls; only x is re-uploaded when it changes.
"""
import sys
if "/opt/trn_rl_repo" not in sys.path:
    sys.path.insert(0, "/opt/trn_rl_repo")
import numpy as np
import ml_dtypes
import concourse.bass as bass
import concourse.bacc as bacc
import concourse.mybir as mybir
import concourse.tile as tile

try:
    import jax as _jax
    _jax.config.update("jax_compilation_cache_dir",
                       "/tmp/gat_jax_compile_cache")
    _jax.config.update("jax_persistent_cache_min_compile_time_secs", 1.0)
    _jax.config.update("jax_persistent_cache_min_entry_size_bytes", 0)
except Exception:
    pass

P = 128
F32 = mybir.dt.float32
BF16 = mybir.dt.bfloat16
FP8 = mybir.dt.float8e4
I32 = mybir.dt.int32
AF = mybir.ActivationFunctionType
ALU = mybir.AluOpType
NPBF16 = ml_dtypes.bfloat16


# ---------------------------------------------------------------------------
# Host-side edge preprocessing (cached across calls keyed on edge_index)
# ---------------------------------------------------------------------------

def prepare(edge_index: np.ndarray, N: int, n_cores: int = 8, nb: int = None):
    E = edge_index.shape[1]
    src0 = edge_index[0].astype(np.int64)
    dst0 = edge_index[1].astype(np.int64)

    nodes_per_core = -(-N // n_cores)
    if nb is None:
        nb = -(-nodes_per_core // P)
    NB = nb
    NL = NB * P

    # degree over non-self edges (self loops handled separately)
    deg = np.bincount(dst0, minlength=N)

    # ---- snake-deal nodes (degree desc) across cores, then blocks ----
    order = np.argsort(-deg, kind="stable")
    pos = np.arange(N)
    rnd = pos // n_cores
    idx = pos % n_cores
    lane = np.where((rnd % 2) == 0, idx, n_cores - 1 - idx)
    node_core = np.empty(N, np.int32)
    node_core[order] = lane.astype(np.int32)

    node_block = np.empty(N, np.int32)
    node_slot = np.empty(N, np.int32)
    for c in range(n_cores):
        nodes_c = order[lane == c]
        m = nodes_c.shape[0]
        assert m <= NB * P, (c, m)
        pos = np.arange(m)
        rnd = pos // NB
        idx = pos % NB
        blk = np.where((rnd % 2) == 0, idx, NB - 1 - idx).astype(np.int32)
        node_block[nodes_c] = blk
        node_slot[nodes_c] = rnd.astype(np.int32)
        assert rnd.max() < P

    new_id = node_core.astype(np.int64) * NL + node_block * P + node_slot
    NROWS = n_cores * NL
    old_of_new = np.full(NROWS, -1, np.int64)
    old_of_new[new_id] = np.arange(N)

    # ---- per-core edge layout (non-self edges only; self loops = chunk 0) ----
    src_new = new_id[src0]
    e_core = node_core[dst0]
    e_block = node_block[dst0]
    e_dslot = node_slot[dst0]

    counts = np.zeros((n_cores, NB), np.int64)
    for c in range(n_cores):
        sel = e_core == c
        counts[c] = np.bincount(e_block[sel], minlength=NB)
    # chunks per block: 1 self chunk + ceil(max_edges/128)
    kb = 1 + np.ceil(counts.max(axis=0) / P).astype(np.int64)
    chunk_block = np.repeat(np.arange(NB), kb)
    NCHUNK = int(kb.sum())
    block_start_chunk = np.concatenate([[0], np.cumsum(kb)])[:-1]

    # n_nodes per (core, block) for self-chunk masks
    nnodes = np.zeros((n_cores, NB), np.int64)
    for c in range(n_cores):
        nnodes[c] = np.bincount(node_block[node_core == c], minlength=NB)

    cores = []
    for c in range(n_cores):
        sel = np.nonzero(e_core == c)[0]
        eb = e_block[sel]
        o = np.argsort(eb, kind="stable")
        sel, eb = sel[o], eb[o]
        starts = np.concatenate([[0], np.cumsum(np.bincount(eb, minlength=NB))])[:-1]
        within = np.arange(sel.shape[0]) - starts[eb]
        slots = (block_start_chunk[eb] + 1) * P + within  # +1: skip self chunk

        slot_src = np.zeros(NCHUNK * P, np.int32)
        slot_d = np.full(NCHUNK * P, -1, np.int32)
        slot_src[slots] = src_new[sel].astype(np.int32)
        slot_d[slots] = e_dslot[sel].astype(np.int32)
        # self chunks: slot p -> node p of block (if present)
        for b in range(NB):
            k0 = block_start_chunk[b]
            nn = nnodes[c, b]
            sl = np.arange(nn)
            slot_d[k0 * P + sl] = sl
            slot_src[k0 * P + sl] = c * NL + b * P + sl  # unused (plain load)

        sd = slot_d.reshape(NCHUNK, P)
        sdT = np.ascontiguousarray(sd.T).astype(np.int32)        # [P, NCHUNK]
        offd = chunk_block[:, None] * P + np.maximum(sd, 0)      # [NCHUNK, P]
        offdT = np.ascontiguousarray(offd.T).astype(np.int32)    # [P, NCHUNK]
        offs_src = slot_src.reshape(NCHUNK, P).T.copy()          # [P, NCHUNK]
        cores.append(dict(offs_src=offs_src, sdT=sdT, offd=offdT))

    return dict(
        cores=cores, new_id=new_id, old_of_new=old_of_new,
        NROWS=NROWS, NCHUNK=NCHUNK, kb=kb, chunk_block=chunk_block,
        block_start_chunk=block_start_chunk,
        node_core=node_core, NB=NB, NL=NL, n_cores=n_cores,
    )


def host_precompute(W1, att_src1, att_dst1, W2, att_src2, att_dst2):
    H, C = att_src1.shape
    Wu = (W1.reshape(-1, H, C) * att_src1[None]).sum(-1)   # [F_in, H]
    Wv = (W1.reshape(-1, H, C) * att_dst1[None]).sum(-1)
    WuWv = np.concatenate([Wu, Wv], axis=1).astype(np.float32)  # [F_in, 16]
    Wu2 = W2 @ att_src2[0]     # [HC]
    Wv2 = W2 @ att_dst2[0]
    Wz2 = np.stack([W2[:, 0], W2[:, 1], Wu2, Wv2], axis=1).astype(np.float32)  # [HC, 4]
    return WuWv, Wz2


# ---------------------------------------------------------------------------
# Fused single-launch Bass kernel
# ---------------------------------------------------------------------------

def build_fused(kb, n_cores: int):
    NB = len(kb)
    NCHUNK = int(np.sum(kb))
    NL = NB * P
    NROWS = n_cores * NL
    T1W = 136  # [h1 (128) | u (8)]

    nc = bacc.Bacc("TRN2", target_bir_lowering=False, debug=False,
                   num_devices=n_cores)
    t_xT = nc.dram_tensor("xT", [P, NL], F32, kind="ExternalInput")
    t_W1 = nc.dram_tensor("W1", [P, P], F32, kind="ExternalInput")
    t_Wuv = nc.dram_tensor("WuWv", [P, 16], F32, kind="ExternalInput")
    t_B1 = nc.dram_tensor("B1", [1, P], F32, kind="ExternalInput")
    t_Wz2 = nc.dram_tensor("Wz2", [P, 4], BF16, kind="ExternalInput")
    t_ID = nc.dram_tensor("ID", [P, P], BF16, kind="ExternalInput")
    t_SD = nc.dram_tensor("SD", [P, NCHUNK], I32, kind="ExternalInput")
    t_OFFS = nc.dram_tensor("OFFS", [P, NCHUNK], I32, kind="ExternalInput")
    t_OFFD = nc.dram_tensor("OFFD", [P, NCHUNK], I32, kind="ExternalInput")
    t_OUT = nc.dram_tensor("OUT", [P, NROWS * 2 // P], BF16, kind="ExternalOutput")

    chunk_block = np.repeat(np.arange(NB), kb)
    csum = np.concatenate([[0], np.cumsum(kb)])
    block_first, block_last = csum[:-1], csum[1:] - 1
    RG = [list(range(n_cores))]

    with tile.TileContext(nc) as tc:
        with tc.tile_pool(name="const", bufs=1) as cpool, \
             tc.tile_pool(name="dram", bufs=1, space="DRAM") as dram:
            W1sb = cpool.tile([P, P], F32, tag="W1sb")
            nc.sync.dma_start(out=W1sb[:], in_=t_W1.ap())
            Wuvsb = cpool.tile([P, 16], F32, tag="Wuvsb")
            nc.sync.dma_start(out=Wuvsb[:], in_=t_Wuv.ap())
            b1row = cpool.tile([1, P], F32, tag="b1row")
            nc.sync.dma_start(out=b1row[:], in_=t_B1.ap())
            Wz2sb = cpool.tile([P, 4], BF16, tag="Wz2sb")
            nc.sync.dma_start(out=Wz2sb[:], in_=t_Wz2.ap())
            idsb = cpool.tile([P, P], BF16, tag="idsb")
            nc.sync.dma_start(out=idsb[:], in_=t_ID.ap())
            sd_t = cpool.tile([P, NCHUNK], I32, tag="sd_t")
            nc.sync.dma_start(out=sd_t[:], in_=t_SD.ap())
            offs_t = cpool.tile([P, NCHUNK], I32, tag="offs_t")
            nc.sync.dma_start(out=offs_t[:], in_=t_OFFS.ap())
            offd_t = cpool.tile([P, NCHUNK], I32, tag="offd_t")
            nc.sync.dma_start(out=offd_t[:], in_=t_OFFD.ap())
            onesb = cpool.tile([1, P], F32, tag="onesb")
            nc.gpsimd.memset(onesb[:], 1.0)
            iota_t = cpool.tile([P, P], I32, tag="iota_t")
            nc.gpsimd.iota(iota_t[:], pattern=[[1, P]], base=0,
                           channel_multiplier=0)
            b1bc = cpool.tile([P, P], BF16, tag="b1bc")

            T1_loc = dram.tile([NL, T1W], BF16, tag="T1_loc")
            T1_full = dram.tile([NROWS, T1W], BF16, addr_space="Shared",
                                tag="T1_full")
            TVd = dram.tile([NL, 8], BF16, tag="TVd")
            Tz_loc = dram.tile([NL, 4], F32, tag="Tz_loc")
            Tz_full = dram.tile([NROWS, 4], F32, addr_space="Shared",
                                tag="Tz_full")
            OUT_loc = dram.tile([NL, 2], F32, tag="OUT_loc")
            OUT_full = dram.tile([NROWS, 2], F32, addr_space="Shared",
                                 tag="OUT_full")

            # ---------------- phase A: node values ----------------
            with tc.tile_pool(name="psB1", bufs=1, space="PSUM") as psb1:
                pb1 = psb1.tile([P, P], F32, tag="pb1")
                nc.tensor.matmul(pb1[:], lhsT=onesb[:], rhs=b1row[:],
                                 start=True, stop=True)
                nc.scalar.activation(b1bc[:], pb1[:], AF.Copy)
            with tc.tile_pool(name="sbA", bufs=3) as sbA, \
                 tc.tile_pool(name="psA", bufs=4, space="PSUM") as psA:
                for t in range(NB):
                    xTt = sbA.tile([P, P], F32, tag="xT")
                    nc.sync.dma_start(out=xTt[:],
                                      in_=t_xT.ap()[:, t * P:(t + 1) * P])
                    ph = psA.tile([P, P], F32, tag="ph")
                    nc.tensor.matmul(ph[:], lhsT=xTt[:], rhs=W1sb[:],
                                     start=True, stop=True)
                    puv = psA.tile([P, 16], F32, tag="puv")
                    nc.tensor.matmul(puv[:], lhsT=xTt[:], rhs=Wuvsb[:],
                                     start=True, stop=True)
                    hrow = sbA.tile([P, T1W], BF16, tag="hrow")
                    nc.scalar.activation(hrow[:, 0:P], ph[:], AF.Copy)
                    nc.vector.tensor_copy(hrow[:, P:P + 8], puv[:, 0:8])
                    vrow = sbA.tile([P, 8], BF16, tag="vrow")
                    nc.vector.tensor_copy(vrow[:], puv[:, 8:16])
                    nc.sync.dma_start(out=T1_loc[t * P:(t + 1) * P, :],
                                      in_=hrow[:])
                    nc.sync.dma_start(out=TVd[t * P:(t + 1) * P, :],
                                      in_=vrow[:])

            nc.gpsimd.collective_compute(
                "AllGather", ALU.bypass, replica_groups=RG,
                ins=[T1_loc.opt()], outs=[T1_full.opt()])

            # ---------------- phase B: layer-1 edges ----------------
            GW = 8
            with tc.tile_pool(name="gatB", bufs=3) as gat, \
                 tc.tile_pool(name="winB", bufs=3) as win, \
                 tc.tile_pool(name="blkB", bufs=3) as blk, \
                 tc.tile_pool(name="accB", bufs=4, space="PSUM") as accp, \
                 tc.tile_pool(name="ps2B", bufs=2, space="PSUM") as ps2:
                psum_of_block = {}
                for w0 in range(0, NCHUNK, GW):
                    w1 = min(w0 + GW, NCHUNK)
                    nw = w1 - w0
                    s_gen = win.tile([P, GW * P], BF16, tag="sg")
                    nc.vector.tensor_tensor(
                        s_gen[:, 0:nw * P].rearrange("p (w d) -> p w d", d=P),
                        sd_t[:, w0:w1].unsqueeze(2).to_broadcast([P, nw, P]),
                        iota_t[:].unsqueeze(1).to_broadcast([P, nw, P]),
                        op=ALU.is_equal)
                    g = gat.tile([P, GW, T1W], BF16, tag="g")
                    vg = gat.tile([P, GW, 8], BF16, tag="vg")
                    for k in range(w0, w1):
                        b = chunk_block[k]
                        if k == block_first[b]:
                            nc.sync.dma_start(
                                out=g[:, k - w0, :],
                                in_=T1_loc[b * P:(b + 1) * P, :])
                            nc.sync.dma_start(
                                out=vg[:, k - w0, :],
                                in_=TVd[b * P:(b + 1) * P, :])
                        else:
                            nc.gpsimd.indirect_dma_start(
                                out=g[:, k - w0, :], out_offset=None,
                                in_=T1_full[:],
                                in_offset=bass.IndirectOffsetOnAxis(
                                    ap=offs_t[:, k:k + 1], axis=0))
                            nc.gpsimd.indirect_dma_start(
                                out=vg[:, k - w0, :], out_offset=None,
                                in_=TVd[:],
                                in_offset=bass.IndirectOffsetOnAxis(
                                    ap=offd_t[:, k:k + 1], axis=0))
                    # e = u + v ; lrelu ; exp
                    e_t = win.tile([P, GW, 8], F32, tag="e")
                    nc.vector.tensor_add(
                        e_t[:, 0:nw, :], g[:, 0:nw, P:P + 8], vg[:, 0:nw, :])
                    nc.vector.scalar_tensor_tensor(
                        e_t[:, 0:nw, :], in0=e_t[:, 0:nw, :], scalar=0.2,
                        in1=e_t[:, 0:nw, :], op0=ALU.mult, op1=ALU.max)
                    ex_t = win.tile([P, GW, 8], BF16, tag="ex")
                    nc.scalar.activation(ex_t[:, 0:nw, :], e_t[:, 0:nw, :],
                                         AF.Exp)
                    m_t = win.tile([P, GW, T1W], BF16, tag="m")
                    exb = ex_t[:, 0:nw, :].unsqueeze(3).to_broadcast(
                        [P, nw, 8, 16])
                    nc.vector.tensor_mul(
                        m_t[:, 0:nw, 0:P].rearrange(
                            "p w (h c) -> p w h c", c=16),
                        g[:, 0:nw, 0:P].rearrange(
                            "p w (h c) -> p w h c", c=16),
                        exb)
                    nc.vector.tensor_copy(m_t[:, 0:nw, P:P + 8],
                                          ex_t[:, 0:nw, :])
                    for k in range(w0, w1):
                        b = chunk_block[k]
                        if k == block_first[b]:
                            psum_of_block[b] = accp.tile(
                                [P, T1W], F32, tag="acc", name=f"acc{b}")
                        nc.tensor.matmul(
                            psum_of_block[b][:],
                            lhsT=s_gen[:, (k - w0) * P:(k - w0 + 1) * P],
                            rhs=m_t[:, k - w0, :],
                            start=(k == block_first[b]),
                            stop=(k == block_last[b]))
                        if k == block_last[b]:
                            acc = psum_of_block.pop(b)
                            _finish_block_B(nc, blk, ps2, acc, Wz2sb, idsb,
                                            b1bc, Tz_loc, b)

            nc.gpsimd.collective_compute(
                "AllGather", ALU.bypass, replica_groups=RG,
                ins=[Tz_loc.opt()], outs=[Tz_full.opt()])

            # ---------------- phase C: layer-2 edges ----------------
            GW2 = 16
            with tc.tile_pool(name="gatC", bufs=3) as gat, \
                 tc.tile_pool(name="winC", bufs=3) as win, \
                 tc.tile_pool(name="blkC", bufs=3) as blk, \
                 tc.tile_pool(name="accC", bufs=4, space="PSUM") as accp:
                psum_of_block = {}
                for w0 in range(0, NCHUNK, GW2):
                    w1 = min(w0 + GW2, NCHUNK)
                    nw = w1 - w0
                    s_gen = win.tile([P, GW2 * P], BF16, tag="sg")
                    nc.vector.tensor_tensor(
                        s_gen[:, 0:nw * P].rearrange("p (w d) -> p w d", d=P),
                        sd_t[:, w0:w1].unsqueeze(2).to_broadcast([P, nw, P]),
                        iota_t[:].unsqueeze(1).to_broadcast([P, nw, P]),
                        op=ALU.is_equal)
                    zg = gat.tile([P, GW2, 4], F32, tag="zg")
                    vg2 = gat.tile([P, GW2, 4], F32, tag="vg2")
                    for k in range(w0, w1):
                        b = chunk_block[k]
                        if k == block_first[b]:
                            nc.sync.dma_start(
                                out=zg[:, k - w0, :],
                                in_=Tz_loc[b * P:(b + 1) * P, :])
                            nc.sync.dma_start(
                                out=vg2[:, k - w0, :],
                                in_=Tz_loc[b * P:(b + 1) * P, :])
                        else:
                            nc.gpsimd.indirect_dma_start(
                                out=zg[:, k - w0, :], out_offset=None,
                                in_=Tz_full[:],
                                in_offset=bass.IndirectOffsetOnAxis(
                                    ap=offs_t[:, k:k + 1], axis=0))
                            nc.gpsimd.indirect_dma_start(
                                out=vg2[:, k - w0, :], out_offset=None,
                                in_=Tz_loc[:],
                                in_offset=bass.IndirectOffsetOnAxis(
                                    ap=offd_t[:, k:k + 1], axis=0))
                    e_t = win.tile([P, GW2], F32, tag="e")
                    nc.vector.tensor_add(e_t[:, 0:nw], zg[:, 0:nw, 2],
                                         vg2[:, 0:nw, 3])
                    nc.vector.scalar_tensor_tensor(
                        e_t[:, 0:nw], in0=e_t[:, 0:nw], scalar=0.2,
                        in1=e_t[:, 0:nw], op0=ALU.mult, op1=ALU.max)
                    ex_t = win.tile([P, GW2], F32, tag="ex")
                    nc.scalar.activation(ex_t[:, 0:nw], e_t[:, 0:nw], AF.Exp)
                    m_t = win.tile([P, GW2, 3], BF16, tag="m")
                    nc.vector.tensor_mul(
                        m_t[:, 0:nw, 0:2], zg[:, 0:nw, 0:2],
                        ex_t[:, 0:nw].unsqueeze(2).to_broadcast([P, nw, 2]))
                    nc.vector.tensor_copy(m_t[:, 0:nw, 2], ex_t[:, 0:nw])
                    for k in range(w0, w1):
                        b = chunk_block[k]
                        if k == block_first[b]:
                            psum_of_block[b] = accp.tile(
                                [P, 3], F32, tag="acc", name=f"acc{b}")
                        nc.tensor.matmul(
                            psum_of_block[b][:],
                            lhsT=s_gen[:, (k - w0) * P:(k - w0 + 1) * P],
                            rhs=m_t[:, k - w0, :],
                            start=(k == block_first[b]),
                            stop=(k == block_last[b]))
                        if k == block_last[b]:
                            acc = psum_of_block.pop(b)
                            s_eps = blk.tile([P, 1], F32, tag="seps")
                            nc.vector.tensor_scalar_add(s_eps[:],
                                                        acc[:, 2:3], 1e-16)
                            rcp = blk.tile([P, 1], F32, tag="rcp")
                            nc.vector.reciprocal(rcp[:], s_eps[:])
                            orow = blk.tile([P, 2], F32, tag="orow")
                            nc.vector.tensor_mul(orow[:], acc[:, 0:2],
                                                 rcp[:].to_broadcast([P, 2]))
                            nc.sync.dma_start(
                                out=OUT_loc[b * P:(b + 1) * P, :],
                                in_=orow[:])

            nc.gpsimd.collective_compute(
                "AllGather", ALU.bypass, replica_groups=RG,
                ins=[OUT_loc.opt()], outs=[OUT_full.opt()])
            # cast f32 -> bf16 through SBUF to halve the D2H payload
            QF = NROWS * 2 // P
            with tc.tile_pool(name="cast", bufs=2) as cast:
                of32 = cast.tile([P, QF], F32, tag="of32")
                nc.sync.dma_start(
                    out=of32[:],
                    in_=OUT_full.rearrange("(p q) c -> p (q c)", p=P))
                obf = cast.tile([P, QF], BF16, tag="obf")
                nc.vector.tensor_copy(obf[:], of32[:])
                nc.sync.dma_start(out=t_OUT.ap(), in_=obf[:])
    nc.compile()
    return nc


def _finish_block_B(nc, blk, ps2, acc, Wz2sb, idsb, b1bc, Tz_loc, b):
    s_eps = blk.tile([P, 8], F32, tag="seps")
    nc.vector.tensor_scalar_add(s_eps[:], acc[:, P:P + 8], 1e-16)
    rcp = blk.tile([P, 8], F32, tag="rcp")
    nc.vector.reciprocal(rcp[:], s_eps[:])
    h2 = blk.tile([P, P], BF16, tag="h2")
    nc.vector.tensor_mul(
        h2[:].rearrange("p (h c) -> p h c", c=16),
        acc[:, 0:P].rearrange("p (h c) -> p h c", c=16),
        rcp[:].unsqueeze(2).to_broadcast([P, 8, 16]))
    nc.vector.tensor_add(h2[:], h2[:], b1bc[:])
    # ELU = max(h2,0) + min(exp(h2)-1, 0)
    ex1 = blk.tile([P, P], BF16, tag="elu1")
    nc.scalar.activation(ex1[:], h2[:], AF.Exp)
    nc.vector.tensor_scalar(ex1[:], in0=ex1[:], scalar1=-1.0, scalar2=0.0,
                            op0=ALU.add, op1=ALU.min)
    h2e = blk.tile([P, P], BF16, tag="h2e")
    nc.vector.scalar_tensor_tensor(h2e[:], in0=h2[:], scalar=0.0, in1=ex1[:],
                                   op0=ALU.max, op1=ALU.add)
    pt = ps2.tile([P, P], BF16, tag="ps2")
    nc.tensor.transpose(pt[:], h2e[:], idsb[:])
    h2T = blk.tile([P, P], BF16, tag="h2T")
    nc.scalar.activation(h2T[:], pt[:], AF.Copy)
    pz = ps2.tile([P, 4], F32, tag="ps2")
    nc.tensor.matmul(pz[:], lhsT=h2T[:], rhs=Wz2sb[:], start=True, stop=True)
    zrow = blk.tile([P, 4], F32, tag="zrow")
    nc.vector.tensor_copy(zrow[:], pz[:])
    nc.sync.dma_start(out=Tz_loc[b * P:(b + 1) * P, :], in_=zrow[:])


# ---------------------------------------------------------------------------
# Persistent PJRT runner: compile once, keep constants device-resident
# ---------------------------------------------------------------------------

class FusedRunner:
    def __init__(self, nc, n_cores: int):
        import jax
        from jax.sharding import Mesh, NamedSharding, PartitionSpec
        from jax.experimental.shard_map import shard_map
        from concourse import bass2jax
        bass2jax.install_neuronx_cc_hook()
        self.jax = jax
        self.nc = nc
        self.n_cores = n_cores

        pid_name = (nc.partition_id_tensor.name
                    if nc.partition_id_tensor is not None else None)
        in_names, out_names, out_avals = [], [], []
        for alloc in nc.m.functions[0].allocations:
            if not isinstance(alloc, mybir.MemoryLocationSet):
                continue
            name = alloc.memorylocations[0].name
            if alloc.kind == "ExternalInput":
                if name != pid_name:
                    in_names.append(name)
            elif alloc.kind == "ExternalOutput":
                assert alloc.tensor_shape is not None
                out_names.append(name)
                out_avals.append(jax.core.ShapedArray(
                    tuple(alloc.tensor_shape), mybir.dt.np(alloc.dtype)))
        self.in_names = list(in_names)
        self.out_names = out_names

        all_names = list(in_names) + list(out_names)
        if pid_name is not None:
            all_names.append(pid_name)
        dbg_name = nc.dbg_addr.name if nc.dbg_addr is not None else None
        if dbg_name is not None and dbg_name in self.in_names:
            raise RuntimeError("debug kernels not supported by FusedRunner")

        def _body(*args):
            operands = list(args)
            if pid_name is not None:
                operands.append(bass2jax.partition_id_tensor())
            outs = bass2jax._bass_exec_p.bind(
                *operands,
                out_avals=tuple(out_avals),
                in_names=tuple(all_names),
                out_names=tuple(out_names),
                lowering_input_output_aliases=(),
                sim_require_finite=False,
                sim_require_nnan=False,
                nc=nc,
            )
            return tuple(outs)

        devices = jax.devices()[:n_cores]
        assert len(devices) == n_cores
        self.mesh = Mesh(np.asarray(devices), ("core",))
        self.sharding = NamedSharding(self.mesh, PartitionSpec("core"))
        n_args = len(in_names) + len(out_avals)
        in_specs = (PartitionSpec("core"),) * n_args
        out_specs = (PartitionSpec("core"),) * len(out_names)
        self._fn = jax.jit(shard_map(
            _body, mesh=self.mesh, in_specs=in_specs, out_specs=out_specs,
            check_rep=False))
        # persistent (never-donated) zero buffers for the output params
        self._zeros = [
            jax.device_put(
                np.zeros((n_cores * aval.shape[0], *aval.shape[1:]),
                         aval.dtype), self.sharding)
            for aval in out_avals
        ]
        # AOT-compile with the bass effect suppressed (C++ fast-path
        # dispatch); fall back to the plain jit on any failure.
        in_avals = []
        for alloc in nc.m.functions[0].allocations:
            if not isinstance(alloc, mybir.MemoryLocationSet):
                continue
            name = alloc.memorylocations[0].name
            if alloc.kind == "ExternalInput" and name in self.in_names:
                in_avals.append((tuple(alloc.tensor_shape),
                                 mybir.dt.np(alloc.dtype)))
        try:
            sds = [jax.ShapeDtypeStruct((n_cores * s[0], *s[1:]), dt,
                                        sharding=self.sharding)
                   for s, dt in in_avals]
            sds += [jax.ShapeDtypeStruct(z.shape, z.dtype,
                                         sharding=self.sharding)
                    for z in self._zeros]
            from concourse.bass2jax import fast_dispatch_compile
            self._fast = fast_dispatch_compile(
                lambda: self._fn.lower(*sds).compile())
        except Exception:
            self._fast = None
        self.bufs = {}

    def put(self, name: str, per_core: list):
        """Upload per-core arrays (list of n_cores ndarrays) once."""
        glob = np.concatenate([np.ascontiguousarray(a) for a in per_core], axis=0)
        self.bufs[name] = self.jax.device_put(glob, self.sharding)

    def dispatch(self):
        """Async launch; returns output jax arrays (futures)."""
        args = [self.bufs[nm] for nm in self.in_names] + self._zeros
        if self._fast is not None:
            try:
                return self._fast(*args)
            except Exception:
                self._fast = None
        return self._fn(*args)

    def fetch(self, outs):
        """Fetch outputs; OUT is replicated across cores, so pull only the
        first shard (one D2H round trip)."""
        res = {}
        for nm, arr in zip(self.out_names, outs):
            s0 = arr.addressable_shards[0]
            try:
                s0.data.copy_to_host_async()
            except Exception:
                pass
            res[nm] = np.asarray(s0.data)
        return res

    def run(self):
        return self.fetch(self.dispatch())


# ---------------------------------------------------------------------------
# kernel() entry with content-addressed caching
# ---------------------------------------------------------------------------

_STATE = {}


def _same(a, b):
    return (b is not None and a.shape == b.shape and a.dtype == b.dtype
            and np.array_equal(a, b))


def _kernel_impl(x, edge_index, W1, att_src1, att_dst1, b1, W2, att_src2, att_dst2, b2):
    x = np.ascontiguousarray(np.asarray(x, dtype=np.float32))
    edge_index = np.ascontiguousarray(np.asarray(edge_index))
    W1 = np.asarray(W1, np.float32); W2 = np.asarray(W2, np.float32)
    att_src1 = np.asarray(att_src1, np.float32)
    att_dst1 = np.asarray(att_dst1, np.float32)
    att_src2 = np.asarray(att_src2, np.float32)
    att_dst2 = np.asarray(att_dst2, np.float32)
    b1 = np.asarray(b1, np.float32); b2 = np.asarray(b2, np.float32)
    N = x.shape[0]
    n_cores = 8

    st = _STATE
    # Optimistic dispatch: reuse the run pre-launched at the end of the
    # previous call if there is one, else launch now with the currently
    # device-resident inputs; verify cache hits while it runs.
    spec_outs = None
    if "runner" in st:
        try:
            spec_outs = st["runner"].dispatch()
        except Exception:
            spec_outs = None

    # --- edge-structure cache (masks, offsets, runner) ---
    if not (_same(edge_index, st.get("edges")) and st.get("N") == N):
        spec_outs = None
        pr = prepare(edge_index, N, n_cores)
        nc = build_fused(pr["kb"], n_cores)
        runner = FusedRunner(nc, n_cores)
        runner.put("SD", [co["sdT"] for co in pr["cores"]])
        runner.put("OFFS", [co["offs_src"] for co in pr["cores"]])
        runner.put("OFFD", [co["offd"] for co in pr["cores"]])
        st.clear()
        st.update(edges=edge_index.copy(), N=N, pr=pr, runner=runner,
                  weights=None, x=None)
    pr, runner = st["pr"], st["runner"]

    # --- weight cache ---
    wpack = np.concatenate([W1.ravel(), att_src1.ravel(), att_dst1.ravel(),
                            b1.ravel(), W2.ravel(), att_src2.ravel(),
                            att_dst2.ravel(), b2.ravel()])
    if not _same(wpack, st.get("weights")):
        spec_outs = None
        WuWv, Wz2 = host_precompute(W1, att_src1, att_dst1, W2, att_src2,
                                    att_dst2)
        ident = np.eye(P, dtype=NPBF16)
        runner.put("W1", [W1.astype(np.float32)] * n_cores)
        runner.put("WuWv", [WuWv] * n_cores)
        runner.put("B1", [b1.reshape(1, P).astype(np.float32)] * n_cores)
        runner.put("Wz2", [Wz2.astype(NPBF16)] * n_cores)
        runner.put("ID", [ident] * n_cores)
        st["weights"] = wpack.copy()
        st["b2"] = b2.copy()

    # --- x cache ---
    if not _same(x, st.get("x")):
        spec_outs = None
        NL, NROWS = pr["NL"], pr["NROWS"]
        xg = np.zeros((NROWS, x.shape[1]), np.float32)
        xg[pr["new_id"]] = x
        xTs = [np.ascontiguousarray(xg[c * NL:(c + 1) * NL].T)
               for c in range(n_cores)]
        runner.put("xT", xTs)
        st["x"] = x.copy()

    if spec_outs is None:
        spec_outs = runner.dispatch()
    try:
        outs = runner.fetch(spec_outs)
    except Exception:
        # transient relay error: short retry; harder failures escalate to
        # the subprocess fallback in kernel().
        import time as _time
        _time.sleep(1.0)
        outs = runner.fetch(runner.dispatch())
    out_full = np.asarray(outs["OUT"], dtype=np.float32).reshape(pr["NROWS"], 2)
    out = out_full[pr["new_id"]]
    return np.ascontiguousarray((out + st["b2"][None, :]).astype(np.float32))


# ---------------------------------------------------------------------------
# Subprocess fallback: if the axon/PJRT session wedges (e.g. device
# NRT_EXEC_UNIT_UNRECOVERABLE at startup), a fresh process gets a fresh relay
# session. A persistent worker amortizes its compile across calls.
# ---------------------------------------------------------------------------

import os as _os
import subprocess as _subprocess
import tempfile as _tempfile
import time as _time

_MODDIR = _os.path.dirname(_os.path.abspath(__file__))
_WORKER = {}


def _worker_alive():
    w = _WORKER.get("proc")
    return w is not None and w.poll() is None


def _spawn_worker():
    td = _tempfile.mkdtemp(prefix="gat_worker_")
    code = (
        "import sys, os\n"
        f"sys.path.insert(0, {_MODDIR!r})\n"
        "os.environ['GAT_KERNEL_SUBPROC'] = '1'\n"
        "import numpy as np\n"
        "import kernel\n"
        f"td = {td!r}\n"
        "for line in sys.stdin:\n"
        "    if line.strip() != 'GO':\n"
        "        continue\n"
        "    try:\n"
        "        d = np.load(os.path.join(td, 'in.npz'))\n"
        "        out = kernel.kernel(**{k: d[k] for k in d.files})\n"
        "        np.save(os.path.join(td, 'out.npy'), out)\n"
        "        print('DONE', flush=True)\n"
        "    except Exception as e:\n"
        "        print('ERR ' + repr(e)[:500], flush=True)\n"
    )
    proc = _subprocess.Popen([sys.executable, "-c", code],
                             stdin=_subprocess.PIPE, stdout=_subprocess.PIPE,
                             text=True)
    _WORKER["proc"] = proc
    _WORKER["dir"] = td


def _run_via_worker(kw):
    if not _worker_alive():
        _spawn_worker()
    w, td = _WORKER["proc"], _WORKER["dir"]
    np.savez(_os.path.join(td, "in.npz"), **kw)
    w.stdin.write("GO\n")
    w.stdin.flush()
    line = w.stdout.readline().strip()
    if line != "DONE":
        try:
            w.kill()
        except Exception:
            pass
        _WORKER.pop("proc", None)
        raise RuntimeError(f"gat worker failed: {line!r}")
    return np.load(_os.path.join(td, "out.npy"))


def kernel(x, edge_index, W1, att_src1, att_dst1, b1, W2, att_src2, att_dst2, b2):
    kw = dict(x=x, edge_index=edge_index, W1=W1, att_src1=att_src1,
              att_dst1=att_dst1, b1=b1, W2=W2, att_src2=att_src2,
              att_dst2=att_dst2, b2=b2)
    if _STATE.get("degraded") and not _os.environ.get("GAT_KERNEL_SUBPROC"):
        return _run_via_worker(kw)
    try:
        return _kernel_impl(**kw)
    except Exception:
        if _os.environ.get("GAT_KERNEL_SUBPROC"):
            raise
        # The in-process session is likely wedged; route through a fresh
        # process from now on.
        last = None
        for attempt in range(3):
            try:
                res = _run_via_worker(kw)
                _STATE["degraded"] = True
                return res
            except Exception as e:
                last = e
                _time.sleep(5.0 * (attempt + 1))
        raise last
